# revision 15
# baseline (speedup 1.0000x reference)
"""Additive attention (Bahdanau) kernel for 8 Trainium2 NeuronCores.

Reference computation (per batch b):
    h   = enc_seq @ W_h.T                 [T, H]
    s   = dec_state @ W_s.T               [H]
    e_t = v . tanh(h_t + s)               [T]
    e   = where(mask==0, -1e9, e)
    a   = softmax(e)
    ctx = sum_t a_t * enc_seq[t]          [H]

Sharding: data-parallel over batch B=32 -> 4 batches per core, weights
replicated.  Host-side prep (inside kernel()): per-core shard, transpose
enc_seq to [H, T] (so H lands on SBUF partitions for the W_h matmul) and
cast everything to bf16; the int32 mask becomes an additive f32/bf16 bias.

On-core layout (per batch, T chunked by 512):
    psum_h[o, t] = sum_k W_hT[k*128+p, o] * encT[k*128+p, t]   (16 MMs)
    tanh on ACT with per-partition bias s[o]  -> bf16 SBUF
    e[t] = v . tanh  via MM with lhsT = v column (M=1), output written to
           partition 32*b so the 4 batches occupy distinct SBUF rows
    exp on ACT (no max subtraction needed: |e| <= ~18) with accum_out row sum
    p normalized then DMA-broadcast to all 128 partitions
    ctx via fused DVE tensor_tensor_reduce over the resident encT tiles
"""

import os
import sys
import numpy as np

sys.path.insert(0, "/opt/trn_rl_repo")

import ml_dtypes

B, T, H = 32, 4096, 512
NCORES = 8
BL = B // NCORES          # 4 batches per core
P = 128
KT = H // P               # 4 contraction tiles
OT = H // P               # 4 output tiles
TC = 512                  # t-chunk
NTC = T // TC             # 8 chunks per batch
NEG = -1.0e9

_CACHE = {}


def _build(T=T, NTC=NTC, stage=4):
    import concourse.bass as bass
    import concourse.tile as tile
    from concourse import bacc, mybir
    from contextlib import ExitStack

    f32 = mybir.dt.float32
    bf16 = mybir.dt.bfloat16
    ts = bass.ts
    Alu = mybir.AluOpType
    Act = mybir.ActivationFunctionType

    nc = bacc.Bacc()

    enc_t = nc.declare_dram_parameter("enc_t", [BL, H, T], bf16, isOutput=False)
    maskb = nc.declare_dram_parameter("maskb", [BL, T], bf16, isOutput=False)
    dec_t = nc.declare_dram_parameter("dec_t", [H, BL], bf16, isOutput=False)
    w_ht = nc.declare_dram_parameter("w_ht", [H, H], bf16, isOutput=False)
    w_st = nc.declare_dram_parameter("w_st", [H, H], bf16, isOutput=False)
    v_in = nc.declare_dram_parameter("v_in", [P, KT], bf16, isOutput=False)
    out_e = nc.declare_dram_parameter("out", [BL, H], f32, isOutput=True)

    with tile.TileContext(nc) as tc, ExitStack() as ctx:
        const = ctx.enter_context(tc.tile_pool(name="const", bufs=1))
        encp = ctx.enter_context(tc.tile_pool(name="encp", bufs=BL * NTC))
        tanhp = ctx.enter_context(tc.tile_pool(name="tanhp", bufs=4))
        toutp = ctx.enter_context(tc.tile_pool(name="toutp", bufs=2))
        pbcp = ctx.enter_context(tc.tile_pool(name="pbcp", bufs=2))
        ctxp = ctx.enter_context(tc.tile_pool(name="ctxp", bufs=2))
        dramp = ctx.enter_context(tc.tile_pool(name="dramp", bufs=2, space="DRAM"))
        php = ctx.enter_context(tc.tile_pool(name="php", bufs=4, space="PSUM"))
        pep = ctx.enter_context(tc.tile_pool(name="pep", bufs=2, space="PSUM"))
        psp = ctx.enter_context(tc.tile_pool(name="psp", bufs=1, space="PSUM"))

        # ---- constants / small inputs ----
        w_sb = const.tile([P, KT, H], bf16, tag="w_sb")
        nc.sync.dma_start(w_sb[:], w_ht.rearrange("(k p) o -> p k o", p=P))
        ws_sb = const.tile([P, KT, H], bf16, tag="ws_sb")
        nc.sync.dma_start(ws_sb[:], w_st.rearrange("(k p) o -> p k o", p=P))
        v_sb = const.tile([P, KT], bf16, tag="v_sb")
        nc.sync.dma_start(v_sb[:], v_in[:, :])
        dec_sb = const.tile([P, KT, BL], bf16, tag="dec_sb")
        nc.sync.dma_start(dec_sb[:], dec_t.rearrange("(k p) b -> p k b", p=P))
        mask_sb = const.tile([P, T], bf16, tag="mask_sb")
        for b in range(BL):
            nc.sync.dma_start(mask_sb[32 * b : 32 * b + 1, :], maskb[b : b + 1, :])

        e_sb = const.tile([P, T], f32, tag="e_sb")        # rows 32b: e, then exp(e)
        pnorm = const.tile([P, T], bf16, tag="pnorm")     # rows 32b: normalized p
        sums = const.tile([P, NTC + 2], f32, tag="sums")  # row sums, total, recip
        s_sb = const.tile([P, OT, BL], f32, tag="s_sb")   # s[o] per batch
        out_sb = const.tile([P, BL, OT], f32, tag="out_sb")

        # ---- s = W_s @ dec (tiny) ----
        for o in range(OT):
            ps = psp.tile([P, BL], f32, tag="ps")
            for k in range(KT):
                nc.tensor.matmul(
                    ps[:],
                    ws_sb[:, k, ts(o, P)],
                    dec_sb[:, k, :],
                    start=(k == 0),
                    stop=(k == KT - 1),
                )
            nc.scalar.copy(s_sb[:, o, :], ps[:])

        # ---- main pipeline ----
        dbg = const.tile([P, 512], f32, tag="dbg", name="dbg") if stage < 4 else None
        enc_tiles = {}
        for b in range(BL):
            row = slice(32 * b, 32 * b + 1)
            for tci in range(NTC):
                et = encp.tile([P, KT, TC], bf16, tag="enc_tile")
                nc.sync.dma_start(
                    et[:],
                    enc_t[b].rearrange("(k p) t -> p k t", p=P)[
                        :, :, ts(tci, TC)
                    ],
                )
                enc_tiles[(b, tci)] = et

                pe_t = pep.tile([P, TC], f32, tag="pe")
                for o in range(OT):
                    ph = php.tile([P, TC], f32, tag="ph")
                    for k in range(KT):
                        nc.tensor.matmul(
                            ph[:],
                            w_sb[:, k, ts(o, P)],
                            et[:, k, :],
                            start=(k == 0),
                            stop=(k == KT - 1),
                        )
                    tt = tanhp.tile([P, TC], bf16, tag="tt")
                    nc.scalar.activation(
                        tt[:], ph[:], Act.Tanh, bias=s_sb[:, o, b : b + 1]
                    )
                    if stage < 2:
                        nc.scalar.copy(dbg[:], tt[:])
                        continue
                    nc.tensor.matmul(
                        pe_t[row, :],
                        v_sb[:, o : o + 1],
                        tt[:],
                        start=(o == 0),
                        stop=(o == OT - 1),
                        tile_position=(0, 32 * b),
                        skip_group_check=True,
                    )
                if stage < 2:
                    continue
                # e = pe + maskbias  (into the 32b row of e_sb)
                nc.vector.tensor_add(
                    e_sb[row, ts(tci, TC)], pe_t[row, :], mask_sb[row, ts(tci, TC)]
                )
                # p = exp(e) in place, accumulate chunk sum
                nc.scalar.activation(
                    e_sb[row, ts(tci, TC)],
                    e_sb[row, ts(tci, TC)],
                    Act.Exp,
                    accum_out=sums[row, tci : tci + 1],
                )
                if stage == 2:
                    nc.scalar.copy(dbg[:], e_sb[:, ts(tci, TC)])
            if stage < 3:
                continue

            # ---- softmax finalize + context for batch b ----
            nc.vector.tensor_reduce(
                sums[row, NTC : NTC + 1],
                sums[row, 0:NTC],
                axis=mybir.AxisListType.X,
                op=Alu.add,
            )
            nc.vector.reciprocal(sums[row, NTC + 1 : NTC + 2], sums[row, NTC : NTC + 1])
            nc.vector.tensor_scalar_mul(
                pnorm[row, :], e_sb[row, :], sums[row, NTC + 1 : NTC + 2]
            )
            pd = dramp.tile([1, T], bf16, tag="pd")
            nc.sync.dma_start(pd[:], pnorm[row, :])
            pb = pbcp.tile([P, T], bf16, tag="pb")
            nc.sync.dma_start(pb[:], pd[:].to_broadcast((P, T)))
            if stage < 4:
                nc.scalar.copy(dbg[:], pb[:, 0:512])
                continue

            ca = ctxp.tile([P, OT, NTC], f32, tag="ca")
            for ht in range(KT):
                for tci in range(NTC):
                    to = toutp.tile([P, TC], bf16, tag="to")
                    nc.vector.scalar_tensor_tensor(
                        out=to[:],
                        in0=enc_tiles[(b, tci)][:, ht, :],
                        scalar=1.0,
                        in1=pb[:, ts(tci, TC)],
                        op0=Alu.mult,
                        op1=Alu.mult,
                        accum_out=ca[:, ht, tci : tci + 1],
                    )
            nc.vector.tensor_reduce(
                out_sb[:, b, :], ca[:], axis=mybir.AxisListType.X, op=Alu.add
            )

        if stage >= 4:
            nc.sync.dma_start(out_e.rearrange("b (ht p) -> p b ht", p=P), out_sb[:])
        else:
            nc.sync.dma_start(out_e[:, :], dbg[0:BL, :512])

    nc.finalize()
    return nc


def _prep_in_maps(enc_seq, enc_mask, dec_state, W_h, W_s, v):
    bf = ml_dtypes.bfloat16
    w_ht = np.ascontiguousarray(W_h.T).astype(bf)
    w_st = np.ascontiguousarray(W_s.T).astype(bf)
    v_in = np.ascontiguousarray(v.reshape(KT, P).T).astype(bf)
    in_maps = []
    for c in range(NCORES):
        sl = slice(c * BL, (c + 1) * BL)
        enc_t = np.ascontiguousarray(
            enc_seq[sl].transpose(0, 2, 1)
        ).astype(bf)
        maskb = np.where(enc_mask[sl] == 0, np.float32(NEG), np.float32(0.0)).astype(bf)
        dec_t = np.ascontiguousarray(dec_state[sl].T).astype(bf)
        in_maps.append(
            {
                "enc_t": enc_t,
                "maskb": maskb,
                "dec_t": dec_t,
                "w_ht": w_ht,
                "w_st": w_st,
                "v_in": v_in,
            }
        )
    return in_maps


def _run(inputs, trace=False):
    from concourse.bass_utils import run_bass_kernel_spmd

    if "nc" not in _CACHE:
        _CACHE["nc"] = _build()
    nc = _CACHE["nc"]
    in_maps = _prep_in_maps(**{k: np.asarray(v) for k, v in inputs.items()})
    res = run_bass_kernel_spmd(nc, in_maps, core_ids=list(range(NCORES)), trace=trace)
    out = np.concatenate([res.results[c]["out"] for c in range(NCORES)], axis=0)
    return out.astype(np.float32), res


def kernel(**inputs):
    out, _ = _run(inputs, trace=False)
    return out


# revision 19
# speedup vs baseline: 1.0034x; 1.0034x over previous
"""Additive attention (Bahdanau) kernel for 8 Trainium2 NeuronCores.

Reference computation (per batch b):
    h   = enc_seq @ W_h.T                 [T, H]
    s   = dec_state @ W_s.T               [H]
    e_t = v . tanh(h_t + s)               [T]
    e   = where(mask==0, -1e9, e)
    a   = softmax(e)
    ctx = sum_t a_t * enc_seq[t]          [H]

Sharding: data-parallel over batch B=32 -> 4 batches per core, weights
replicated.  Host-side prep (inside kernel()): per-core shard, transpose
enc_seq to [H, T] (so H lands on SBUF partitions for the W_h matmul) and
cast everything to bf16; the int32 mask becomes an additive f32/bf16 bias.

On-core layout (per batch, T chunked by 512):
    psum_h[o, t] = sum_k W_hT[k*128+p, o] * encT[k*128+p, t]   (16 MMs)
    tanh on ACT with per-partition bias s[o]  -> bf16 SBUF
    e[t] = v . tanh  via MM with lhsT = v column (M=1), output written to
           partition 32*b so the 4 batches occupy distinct SBUF rows
    exp on ACT (no max subtraction needed: |e| <= ~18) with accum_out row sum
    p normalized then DMA-broadcast to all 128 partitions
    ctx via fused DVE tensor_tensor_reduce over the resident encT tiles
"""

import os
import sys
import numpy as np

sys.path.insert(0, "/opt/trn_rl_repo")

import ml_dtypes

B, T, H = 32, 4096, 512
NCORES = 8
BL = B // NCORES          # 4 batches per core
P = 128
KT = H // P               # 4 contraction tiles
OT = H // P               # 4 output tiles
TC = 512                  # t-chunk
NTC = T // TC             # 8 chunks per batch
NEG = -1.0e9

_CACHE = {}


def _build(T=T, NTC=NTC, stage=4):
    import concourse.bass as bass
    import concourse.tile as tile
    from concourse import bacc, mybir
    from contextlib import ExitStack

    f32 = mybir.dt.float32
    bf16 = mybir.dt.bfloat16
    ts = bass.ts
    Alu = mybir.AluOpType
    Act = mybir.ActivationFunctionType

    nc = bacc.Bacc()

    enc_t = nc.declare_dram_parameter("enc_t", [BL, H, T], bf16, isOutput=False)
    maskb = nc.declare_dram_parameter("maskb", [BL, T], bf16, isOutput=False)
    dec_t = nc.declare_dram_parameter("dec_t", [H, BL], bf16, isOutput=False)
    w_ht = nc.declare_dram_parameter("w_ht", [H, H], bf16, isOutput=False)
    w_st = nc.declare_dram_parameter("w_st", [H, H], bf16, isOutput=False)
    v_in = nc.declare_dram_parameter("v_in", [P, KT], bf16, isOutput=False)
    out_e = nc.declare_dram_parameter("out", [BL, H], f32, isOutput=True)

    with tile.TileContext(nc) as tc, ExitStack() as ctx:
        const = ctx.enter_context(tc.tile_pool(name="const", bufs=1))
        encp = ctx.enter_context(tc.tile_pool(name="encp", bufs=8))
        tanhp = ctx.enter_context(tc.tile_pool(name="tanhp", bufs=6))
        toutp = ctx.enter_context(tc.tile_pool(name="toutp", bufs=2))
        erowp = ctx.enter_context(tc.tile_pool(name="erowp", bufs=4))
        pexp = ctx.enter_context(tc.tile_pool(name="pexp", bufs=4))
        pbcp = ctx.enter_context(tc.tile_pool(name="pbcp", bufs=4))
        ctxp = ctx.enter_context(tc.tile_pool(name="ctxp", bufs=2))
        dramp = ctx.enter_context(tc.tile_pool(name="dramp", bufs=4, space="DRAM"))
        php = ctx.enter_context(tc.tile_pool(name="php", bufs=5, space="PSUM"))
        pep = ctx.enter_context(tc.tile_pool(name="pep", bufs=2, space="PSUM"))
        psp = ctx.enter_context(tc.tile_pool(name="psp", bufs=1, space="PSUM"))

        # ---- constants / small inputs ----
        w_sb = const.tile([P, KT, H], bf16, tag="w_sb")
        nc.sync.dma_start(w_sb[:], w_ht.rearrange("(k p) o -> p k o", p=P))
        ws_sb = const.tile([P, KT, H], bf16, tag="ws_sb")
        nc.sync.dma_start(ws_sb[:], w_st.rearrange("(k p) o -> p k o", p=P))
        v_sb = const.tile([P, KT], bf16, tag="v_sb")
        nc.sync.dma_start(v_sb[:], v_in[:, :])
        dec_sb = const.tile([P, KT, BL], bf16, tag="dec_sb")
        nc.sync.dma_start(dec_sb[:], dec_t.rearrange("(k p) b -> p k b", p=P))
        mask_sb = const.tile([P, T], bf16, tag="mask_sb")
        for b in range(BL):
            nc.sync.dma_start(mask_sb[32 * b : 32 * b + 1, :], maskb[b : b + 1, :])

        sums = const.tile([P, NTC + 1], f32, tag="sums")  # rows 32b: chunk sums, total
        s_sb = const.tile([P, OT, BL], f32, tag="s_sb")   # s[o] per batch
        out_sb = const.tile([P, BL, OT], f32, tag="out_sb")

        # ---- s = W_s @ dec (tiny) ----
        for o in range(OT):
            ps = psp.tile([P, BL], f32, tag="ps")
            for k in range(KT):
                nc.tensor.matmul(
                    ps[:],
                    ws_sb[:, k, ts(o, P)],
                    dec_sb[:, k, :],
                    start=(k == 0),
                    stop=(k == KT - 1),
                )
            nc.scalar.copy(s_sb[:, o, :], ps[:])

        # ---- main pipeline ----
        # Per (batch, 512-wide t-chunk): matmul h = W_h @ x, tanh(+s) on ACT,
        # e = v . tanh via M=1 matmul to partition 32b, add mask bias, exp
        # (unnormalized), broadcast the exp row to all partitions via a DRAM
        # bounce, then fused multiply+accumulate of exp(e) * x into the
        # context accumulator.  The softmax denominator is applied once at
        # the very end, so nothing waits for a full batch row.
        for b in range(BL):
            row = slice(32 * b, 32 * b + 1)
            ca = ctxp.tile([P, OT, NTC], f32, tag="ca")
            for tci in range(NTC):
                et = encp.tile([P, KT, TC], bf16, tag="enc_tile")
                nc.sync.dma_start(
                    et[:],
                    enc_t[b].rearrange("(k p) t -> p k t", p=P)[
                        :, :, ts(tci, TC)
                    ],
                )

                pe_t = pep.tile([P, TC], f32, tag="pe")
                for o in range(OT):
                    ph = php.tile([P, TC], f32, tag="ph")
                    for k in range(KT):
                        nc.tensor.matmul(
                            ph[:],
                            w_sb[:, k, ts(o, P)],
                            et[:, k, :],
                            start=(k == 0),
                            stop=(k == KT - 1),
                        )
                    tt = tanhp.tile([P, TC], bf16, tag="tt")
                    nc.scalar.activation(
                        tt[:], ph[:], Act.Tanh, bias=s_sb[:, o, b : b + 1]
                    )
                    nc.tensor.matmul(
                        pe_t[row, :],
                        v_sb[:, o : o + 1],
                        tt[:],
                        start=(o == 0),
                        stop=(o == OT - 1),
                        tile_position=(0, 32 * b),
                        skip_group_check=True,
                    )
                # e = pe + maskbias
                erow = erowp.tile([P, TC], f32, tag="erow")
                nc.vector.tensor_add(
                    erow[row, :], pe_t[row, :], mask_sb[row, ts(tci, TC)]
                )
                # p = exp(e) (unnormalized), accumulate chunk sum
                pex = pexp.tile([P, TC], bf16, tag="pex")
                nc.scalar.activation(
                    pex[row, :],
                    erow[row, :],
                    Act.Exp,
                    accum_out=sums[row, tci : tci + 1],
                )
                # broadcast p row to all partitions via DRAM bounce
                pd = dramp.tile([1, TC], bf16, tag="pd")
                nc.sync.dma_start(pd[:], pex[row, :])
                pb = pbcp.tile([P, TC], bf16, tag="pb")
                nc.sync.dma_start(pb[:], pd[:].to_broadcast((P, TC)))
                # ctx_raw[:, ht] += sum_t p[t] * x[t]
                for ht in range(KT):
                    to = toutp.tile([P, TC], bf16, tag="to")
                    nc.vector.scalar_tensor_tensor(
                        out=to[:],
                        in0=et[:, ht, :],
                        scalar=1.0,
                        in1=pb[:, :],
                        op0=Alu.mult,
                        op1=Alu.mult,
                        accum_out=ca[:, ht, tci : tci + 1],
                    )
            # batch row sum and raw context
            nc.vector.tensor_reduce(
                sums[row, NTC : NTC + 1],
                sums[row, 0:NTC],
                axis=mybir.AxisListType.X,
                op=Alu.add,
            )
            nc.vector.tensor_reduce(
                out_sb[:, b, :], ca[:], axis=mybir.AxisListType.X, op=Alu.add
            )

        # ---- final normalization: out_sb /= rowsum (per batch) ----
        # gather the 4 per-batch sums (partitions 0/32/64/96) into one DRAM
        # row, broadcast back to all partitions, reciprocal, multiply.
        sg = dramp.tile([BL, 1], f32, tag="sg")
        for b in range(BL):
            nc.sync.dma_start(
                sg[b : b + 1, :], sums[32 * b : 32 * b + 1, NTC : NTC + 1]
            )
        sb_b = const.tile([P, BL], f32, tag="sb_b")
        nc.sync.dma_start(
            sb_b[:], sg[:].rearrange("b one -> one b").to_broadcast((P, BL))
        )
        rec_b = const.tile([P, BL], f32, tag="rec_b")
        nc.vector.reciprocal(rec_b[:], sb_b[:])
        nc.vector.tensor_mul(
            out_sb[:], out_sb[:], rec_b[:, :, None].to_broadcast((P, BL, OT))
        )
        nc.sync.dma_start(out_e.rearrange("b (ht p) -> p b ht", p=P), out_sb[:])

    nc.finalize()
    return nc


def _prep_in_maps(enc_seq, enc_mask, dec_state, W_h, W_s, v):
    bf = ml_dtypes.bfloat16
    w_ht = np.ascontiguousarray(W_h.T).astype(bf)
    w_st = np.ascontiguousarray(W_s.T).astype(bf)
    v_in = np.ascontiguousarray(v.reshape(KT, P).T).astype(bf)
    in_maps = []
    for c in range(NCORES):
        sl = slice(c * BL, (c + 1) * BL)
        enc_t = np.ascontiguousarray(
            enc_seq[sl].transpose(0, 2, 1)
        ).astype(bf)
        maskb = np.where(enc_mask[sl] == 0, np.float32(NEG), np.float32(0.0)).astype(bf)
        dec_t = np.ascontiguousarray(dec_state[sl].T).astype(bf)
        in_maps.append(
            {
                "enc_t": enc_t,
                "maskb": maskb,
                "dec_t": dec_t,
                "w_ht": w_ht,
                "w_st": w_st,
                "v_in": v_in,
            }
        )
    return in_maps


def _run(inputs, trace=False):
    from concourse.bass_utils import run_bass_kernel_spmd

    if "nc" not in _CACHE:
        _CACHE["nc"] = _build()
    nc = _CACHE["nc"]
    in_maps = _prep_in_maps(**{k: np.asarray(v) for k, v in inputs.items()})
    res = run_bass_kernel_spmd(nc, in_maps, core_ids=list(range(NCORES)), trace=trace)
    out = np.concatenate([res.results[c]["out"] for c in range(NCORES)], axis=0)
    return out.astype(np.float32), res


def kernel(**inputs):
    out, _ = _run(inputs, trace=False)
    return out


# revision 23
# speedup vs baseline: 1.0040x; 1.0006x over previous
"""Additive attention (Bahdanau) kernel for 8 Trainium2 NeuronCores.

Reference computation (per batch b):
    h   = enc_seq @ W_h.T                 [T, H]
    s   = dec_state @ W_s.T               [H]
    e_t = v . tanh(h_t + s)               [T]
    e   = where(mask==0, -1e9, e)
    a   = softmax(e)
    ctx = sum_t a_t * enc_seq[t]          [H]

Sharding: data-parallel over batch B=32 -> 4 batches per core, weights
replicated.  Host-side prep (inside kernel()): per-core shard, transpose
enc_seq to [H, T] (so H lands on SBUF partitions for the W_h matmul) and
cast everything to bf16; the int32 mask becomes an additive f32/bf16 bias.

On-core layout (per batch, T chunked by 512):
    psum_h[o, t] = sum_k W_hT[k*128+p, o] * encT[k*128+p, t]   (16 MMs)
    tanh on ACT with per-partition bias s[o]  -> bf16 SBUF
    e[t] = v . tanh  via MM with lhsT = v column (M=1), output written to
           partition 32*b so the 4 batches occupy distinct SBUF rows
    exp on ACT (no max subtraction needed: |e| <= ~18) with accum_out row sum
    p normalized then DMA-broadcast to all 128 partitions
    ctx via fused DVE tensor_tensor_reduce over the resident encT tiles
"""

import os
import sys
import numpy as np

sys.path.insert(0, "/opt/trn_rl_repo")

import ml_dtypes

B, T, H = 32, 4096, 512
NCORES = 8
BL = B // NCORES          # 4 batches per core
P = 128
KT = H // P               # 4 contraction tiles
OT = H // P               # 4 output tiles
TC = 512                  # t-chunk
NTC = T // TC             # 8 chunks per batch
NEG = -1.0e9

_CACHE = {}


def _build(T=T, NTC=NTC, stage=4):
    import concourse.bass as bass
    import concourse.tile as tile
    from concourse import bacc, mybir
    from contextlib import ExitStack

    f32 = mybir.dt.float32
    bf16 = mybir.dt.bfloat16
    ts = bass.ts
    Alu = mybir.AluOpType
    Act = mybir.ActivationFunctionType

    nc = bacc.Bacc()

    enc_t = nc.declare_dram_parameter("enc_t", [BL, H, T], bf16, isOutput=False)
    maskb = nc.declare_dram_parameter("maskb", [BL, T], bf16, isOutput=False)
    dec_t = nc.declare_dram_parameter("dec_t", [H, BL], bf16, isOutput=False)
    w_ht = nc.declare_dram_parameter("w_ht", [H, H], bf16, isOutput=False)
    w_st = nc.declare_dram_parameter("w_st", [H, H], bf16, isOutput=False)
    v_in = nc.declare_dram_parameter("v_in", [P, KT], bf16, isOutput=False)
    out_e = nc.declare_dram_parameter("out", [BL, H], f32, isOutput=True)

    with tile.TileContext(nc) as tc, ExitStack() as ctx:
        const = ctx.enter_context(tc.tile_pool(name="const", bufs=1))
        encp = ctx.enter_context(tc.tile_pool(name="encp", bufs=8))
        tanhp = ctx.enter_context(tc.tile_pool(name="tanhp", bufs=6))
        toutp = ctx.enter_context(tc.tile_pool(name="toutp", bufs=2))
        erowp = ctx.enter_context(tc.tile_pool(name="erowp", bufs=4))
        pexp = ctx.enter_context(tc.tile_pool(name="pexp", bufs=4))
        pbcp = ctx.enter_context(tc.tile_pool(name="pbcp", bufs=4))
        ctxp = ctx.enter_context(tc.tile_pool(name="ctxp", bufs=2))
        dramp = ctx.enter_context(tc.tile_pool(name="dramp", bufs=4, space="DRAM"))
        php = ctx.enter_context(tc.tile_pool(name="php", bufs=5, space="PSUM"))
        pep = ctx.enter_context(tc.tile_pool(name="pep", bufs=2, space="PSUM"))
        psp = ctx.enter_context(tc.tile_pool(name="psp", bufs=1, space="PSUM"))

        # ---- constants / small inputs ----
        w_sb = const.tile([P, KT, H], bf16, tag="w_sb")
        nc.sync.dma_start(w_sb[:], w_ht.rearrange("(k p) o -> p k o", p=P))
        ws_sb = const.tile([P, KT, H], bf16, tag="ws_sb")
        nc.sync.dma_start(ws_sb[:], w_st.rearrange("(k p) o -> p k o", p=P))
        v_sb = const.tile([P, KT], bf16, tag="v_sb")
        nc.sync.dma_start(v_sb[:], v_in[:, :])
        dec_sb = const.tile([P, KT, BL], bf16, tag="dec_sb")
        nc.sync.dma_start(dec_sb[:], dec_t.rearrange("(k p) b -> p k b", p=P))
        mask_sb = const.tile([P, T], bf16, tag="mask_sb")
        for b in range(BL):
            nc.sync.dma_start(mask_sb[32 * b : 32 * b + 1, :], maskb[b : b + 1, :])

        sums = const.tile([P, NTC + 1], f32, tag="sums")  # rows 32b: chunk sums, total
        s_sb = const.tile([P, OT, BL], f32, tag="s_sb")   # s[o] per batch
        out_sb = const.tile([P, BL, OT], f32, tag="out_sb")
        ones_sb = const.tile([P, P], f32, tag="ones_sb")
        nc.any.memset(ones_sb[:], 1.0)

        # ---- s = W_s @ dec (tiny) ----
        for o in range(OT):
            ps = psp.tile([P, BL], f32, tag="ps")
            for k in range(KT):
                nc.tensor.matmul(
                    ps[:],
                    ws_sb[:, k, ts(o, P)],
                    dec_sb[:, k, :],
                    start=(k == 0),
                    stop=(k == KT - 1),
                )
            nc.scalar.copy(s_sb[:, o, :], ps[:])

        # ---- main pipeline ----
        # Per (batch, 512-wide t-chunk): matmul h = W_h @ x, tanh(+s) on ACT,
        # e = v . tanh via M=1 matmul to partition 32b, add mask bias, exp
        # (unnormalized), broadcast the exp row to all partitions via a DRAM
        # bounce, then fused multiply+accumulate of exp(e) * x into the
        # context accumulator.  The softmax denominator is applied once at
        # the very end, so nothing waits for a full batch row.
        for b in range(BL):
            row = slice(32 * b, 32 * b + 1)
            ca = ctxp.tile([P, OT, NTC], f32, tag="ca")
            for tci in range(NTC):
                et = encp.tile([P, KT, TC], bf16, tag="enc_tile")
                nc.sync.dma_start(
                    et[:],
                    enc_t[b].rearrange("(k p) t -> p k t", p=P)[
                        :, :, ts(tci, TC)
                    ],
                )

                pe_t = pep.tile([P, TC], f32, tag="pe")
                for o in range(OT):
                    ph = php.tile([P, TC], f32, tag="ph")
                    for k in range(KT):
                        nc.tensor.matmul(
                            ph[:],
                            w_sb[:, k, ts(o, P)],
                            et[:, k, :],
                            start=(k == 0),
                            stop=(k == KT - 1),
                        )
                    tt = tanhp.tile([P, TC], bf16, tag="tt")
                    nc.scalar.activation(
                        tt[:], ph[:], Act.Tanh, bias=s_sb[:, o, b : b + 1]
                    )
                    nc.tensor.matmul(
                        pe_t[row, :],
                        v_sb[:, o : o + 1],
                        tt[:],
                        start=(o == 0),
                        stop=(o == OT - 1),
                        tile_position=(0, 32 * b),
                        skip_group_check=True,
                    )
                # e = pe + maskbias
                erow = erowp.tile([P, TC], f32, tag="erow")
                nc.vector.tensor_add(
                    erow[row, :], pe_t[row, :], mask_sb[row, ts(tci, TC)]
                )
                # p = exp(e) (unnormalized), accumulate chunk sum
                pex = pexp.tile([P, TC], bf16, tag="pex")
                nc.scalar.activation(
                    pex[row, :],
                    erow[row, :],
                    Act.Exp,
                    accum_out=sums[row, tci : tci + 1],
                )
                # broadcast p row to all partitions via DRAM bounce
                pd = dramp.tile([1, TC], bf16, tag="pd")
                nc.sync.dma_start(pd[:], pex[row, :])
                pb = pbcp.tile([P, TC], bf16, tag="pb")
                nc.sync.dma_start(pb[:], pd[:].to_broadcast((P, TC)))
                # ctx_raw[:, ht] += sum_t p[t] * x[t]
                for ht in range(KT):
                    to = toutp.tile([P, TC], bf16, tag="to")
                    nc.vector.scalar_tensor_tensor(
                        out=to[:],
                        in0=et[:, ht, :],
                        scalar=1.0,
                        in1=pb[:, :],
                        op0=Alu.mult,
                        op1=Alu.mult,
                        accum_out=ca[:, ht, tci : tci + 1],
                    )
            # batch row sum, then broadcast it to all 128 partitions with a
            # K=1 ones-matmul, reciprocal, scale the raw context, DMA out —
            # all per batch, so only the last batch's (tiny) chain is exposed.
            nc.vector.tensor_reduce(
                sums[row, NTC : NTC + 1],
                sums[row, 0:NTC],
                axis=mybir.AxisListType.X,
                op=Alu.add,
            )
            nc.vector.tensor_reduce(
                out_sb[:, b, :], ca[:], axis=mybir.AxisListType.X, op=Alu.add
            )
            psb = pep.tile([P, 1], f32, tag="pe", name="psb")
            nc.tensor.matmul(
                psb[:, :],
                ones_sb[row, :],
                sums[row, NTC : NTC + 1],
                start=True,
                stop=True,
                tile_position=(32 * b, 0),
                skip_group_check=True,
            )
            rec = toutp.tile([P, 1], f32, tag="rec")
            nc.vector.reciprocal(rec[:], psb[:, :])
            nc.vector.tensor_mul(
                out_sb[:, b, :],
                out_sb[:, b, :],
                rec[:].to_broadcast((P, OT)),
            )
            nc.sync.dma_start(
                out_e.rearrange("b (ht p) -> p b ht", p=P)[:, b, :],
                out_sb[:, b, :],
            )

    nc.finalize()
    return nc


def _prep_in_maps(enc_seq, enc_mask, dec_state, W_h, W_s, v):
    bf = ml_dtypes.bfloat16
    w_ht = np.ascontiguousarray(W_h.T).astype(bf)
    w_st = np.ascontiguousarray(W_s.T).astype(bf)
    v_in = np.ascontiguousarray(v.reshape(KT, P).T).astype(bf)
    in_maps = []
    for c in range(NCORES):
        sl = slice(c * BL, (c + 1) * BL)
        enc_t = np.ascontiguousarray(
            enc_seq[sl].transpose(0, 2, 1)
        ).astype(bf)
        maskb = np.where(enc_mask[sl] == 0, np.float32(NEG), np.float32(0.0)).astype(bf)
        dec_t = np.ascontiguousarray(dec_state[sl].T).astype(bf)
        in_maps.append(
            {
                "enc_t": enc_t,
                "maskb": maskb,
                "dec_t": dec_t,
                "w_ht": w_ht,
                "w_st": w_st,
                "v_in": v_in,
            }
        )
    return in_maps


def _run(inputs, trace=False):
    from concourse.bass_utils import run_bass_kernel_spmd

    if "nc" not in _CACHE:
        _CACHE["nc"] = _build()
    nc = _CACHE["nc"]
    in_maps = _prep_in_maps(**{k: np.asarray(v) for k, v in inputs.items()})
    res = run_bass_kernel_spmd(nc, in_maps, core_ids=list(range(NCORES)), trace=trace)
    out = np.concatenate([res.results[c]["out"] for c in range(NCORES)], axis=0)
    return out.astype(np.float32), res


def kernel(**inputs):
    out, _ = _run(inputs, trace=False)
    return out


# revision 38
# speedup vs baseline: 1.0292x; 1.0251x over previous
"""Additive attention (Bahdanau) kernel for 8 Trainium2 NeuronCores.

Reference computation (per batch b):
    h   = enc_seq @ W_h.T                 [T, H]
    s   = dec_state @ W_s.T               [H]
    e_t = v . tanh(h_t + s)               [T]
    e   = where(mask==0, -1e9, e)
    a   = softmax(e)
    ctx = sum_t a_t * enc_seq[t]          [H]

Sharding: data-parallel over batch B=32 -> 4 batches per core, weights
replicated.  Host-side prep (inside kernel()): per-core shard, transpose
enc_seq to [H, T] (so H lands on SBUF partitions for the W_h matmul) and
cast everything to bf16; the int32 mask becomes an additive f32/bf16 bias.

On-core layout (per batch, T chunked by 512):
    psum_h[o, t] = sum_k W_hT[k*128+p, o] * encT[k*128+p, t]   (16 MMs)
    tanh on ACT with per-partition bias s[o]  -> bf16 SBUF
    e[t] = v . tanh  via MM with lhsT = v column (M=1), output written to
           partition 32*b so the 4 batches occupy distinct SBUF rows
    exp on ACT (no max subtraction needed: |e| <= ~18) with accum_out row sum
    unnormalized p DMA-broadcast to all 128 partitions via a DRAM bounce
    ctx_raw accumulated per chunk with fused DVE scalar_tensor_tensor;
    the softmax denominator is divided out once per batch at the end
"""

import os
import sys
import numpy as np

sys.path.insert(0, "/opt/trn_rl_repo")

import ml_dtypes

B, T, H = 32, 4096, 512
NCORES = 8
BL = B // NCORES          # 4 batches per core
P = 128
KT = H // P               # 4 contraction tiles
OT = H // P               # 4 output tiles
TC = 512                  # t-chunk
NTC = T // TC             # 8 chunks per batch
NEG = -1.0e9

_CACHE = {}


def _build(T=T, NTC=NTC, stage=4):
    import concourse.bass as bass
    import concourse.tile as tile
    from concourse import bacc, mybir
    from contextlib import ExitStack

    f32 = mybir.dt.float32
    bf16 = mybir.dt.bfloat16
    ts = bass.ts
    Alu = mybir.AluOpType
    Act = mybir.ActivationFunctionType

    nc = bacc.Bacc()

    enc_t = nc.declare_dram_parameter("enc_t", [BL, H, T], bf16, isOutput=False)
    maskb = nc.declare_dram_parameter("maskb", [BL, T], bf16, isOutput=False)
    dec_t = nc.declare_dram_parameter("dec_t", [H, BL], bf16, isOutput=False)
    w_ht = nc.declare_dram_parameter("w_ht", [H, H], bf16, isOutput=False)
    w_st = nc.declare_dram_parameter("w_st", [H, H], bf16, isOutput=False)
    v_in = nc.declare_dram_parameter("v_in", [P, KT], bf16, isOutput=False)
    out_e = nc.declare_dram_parameter("out", [BL, H], f32, isOutput=True)

    with tile.TileContext(nc) as tc, ExitStack() as ctx:
        const = ctx.enter_context(tc.tile_pool(name="const", bufs=1))
        encp = ctx.enter_context(tc.tile_pool(name="encp", bufs=8))
        tanhp = ctx.enter_context(tc.tile_pool(name="tanhp", bufs=6))
        toutp = ctx.enter_context(tc.tile_pool(name="toutp", bufs=2))
        erowp = ctx.enter_context(tc.tile_pool(name="erowp", bufs=4))
        pexp = ctx.enter_context(tc.tile_pool(name="pexp", bufs=4))
        pbcp = ctx.enter_context(tc.tile_pool(name="pbcp", bufs=4))
        ctxp = ctx.enter_context(tc.tile_pool(name="ctxp", bufs=2))
        dramp = ctx.enter_context(tc.tile_pool(name="dramp", bufs=4, space="DRAM"))
        php = ctx.enter_context(tc.tile_pool(name="php", bufs=5, space="PSUM"))
        pep = ctx.enter_context(tc.tile_pool(name="pep", bufs=2, space="PSUM"))
        psp = ctx.enter_context(tc.tile_pool(name="psp", bufs=1, space="PSUM"))

        # ---- constants / small inputs ----
        w_sb = const.tile([P, KT, H], bf16, tag="w_sb")
        nc.sync.dma_start(w_sb[:], w_ht.rearrange("(k p) o -> p k o", p=P))
        ws_sb = const.tile([P, KT, H], bf16, tag="ws_sb")
        nc.sync.dma_start(ws_sb[:], w_st.rearrange("(k p) o -> p k o", p=P))
        v_sb = const.tile([P, KT], bf16, tag="v_sb")
        nc.sync.dma_start(v_sb[:], v_in[:, :])
        dec_sb = const.tile([P, KT, BL], bf16, tag="dec_sb")
        nc.sync.dma_start(dec_sb[:], dec_t.rearrange("(k p) b -> p k b", p=P))
        mask_sb = const.tile([P, T], bf16, tag="mask_sb")
        for b in range(BL):
            nc.sync.dma_start(mask_sb[32 * b : 32 * b + 1, :], maskb[b : b + 1, :])

        sums = const.tile([P, NTC + 1], f32, tag="sums")  # rows 32b: chunk sums, total
        s_sb = const.tile([P, OT, BL], f32, tag="s_sb")   # s[o] per batch
        out_sb = const.tile([P, BL, OT], f32, tag="out_sb")
        ones_sb = const.tile([P, P], f32, tag="ones_sb")
        nc.any.memset(ones_sb[:], 1.0)

        # ---- s = W_s @ dec (tiny) ----
        for o in range(OT):
            ps = psp.tile([P, BL], f32, tag="ps")
            for k in range(KT):
                nc.tensor.matmul(
                    ps[:],
                    ws_sb[:, k, ts(o, P)],
                    dec_sb[:, k, :],
                    start=(k == 0),
                    stop=(k == KT - 1),
                )
            nc.scalar.copy(s_sb[:, o, :], ps[:])

        # ---- main pipeline ----
        # Per (batch, 512-wide t-chunk): matmul h = W_h @ x, tanh(+s) on ACT,
        # e = v . tanh via M=1 matmul to partition 32b, add mask bias, exp
        # (unnormalized), broadcast the exp row to all partitions via a DRAM
        # bounce, then fused multiply+accumulate of exp(e) * x into the
        # context accumulator.  The softmax denominator is applied once at
        # the very end, so nothing waits for a full batch row.
        for b in range(BL):
            row = slice(32 * b, 32 * b + 1)
            ca = ctxp.tile([P, OT, NTC], f32, tag="ca")
            for tci in range(NTC):
                et = encp.tile([P, KT, TC], bf16, tag="enc_tile")
                nc.sync.dma_start(
                    et[:],
                    enc_t[b].rearrange("(k p) t -> p k t", p=P)[
                        :, :, ts(tci, TC)
                    ],
                )

                pe_t = pep.tile([P, TC], f32, tag="pe")
                for o in range(OT):
                    ph = php.tile([P, TC], f32, tag="ph")
                    for k in range(KT):
                        nc.tensor.matmul(
                            ph[:],
                            w_sb[:, k, ts(o, P)],
                            et[:, k, :],
                            start=(k == 0),
                            stop=(k == KT - 1),
                        )
                    tt = tanhp.tile([P, TC], bf16, tag="tt")
                    nc.scalar.activation(
                        tt[:], ph[:], Act.Tanh, bias=s_sb[:, o, b : b + 1]
                    )
                    nc.tensor.matmul(
                        pe_t[row, :],
                        v_sb[:, o : o + 1],
                        tt[:],
                        start=(o == 0),
                        stop=(o == OT - 1),
                        tile_position=(0, 32 * b),
                        skip_group_check=True,
                    )
                # e = pe + maskbias
                erow = erowp.tile([P, TC], f32, tag="erow")
                nc.vector.tensor_add(
                    erow[row, :], pe_t[row, :], mask_sb[row, ts(tci, TC)]
                )
                # p = exp(e) (unnormalized), accumulate chunk sum
                pex = pexp.tile([P, TC], bf16, tag="pex")
                nc.scalar.activation(
                    pex[row, :],
                    erow[row, :],
                    Act.Exp,
                    accum_out=sums[row, tci : tci + 1],
                )
                # broadcast p row to all partitions via a DRAM bounce
                pd = dramp.tile([1, TC], bf16, tag="pd")
                nc.sync.dma_start(pd[:], pex[row, :])
                pb = pbcp.tile([P, TC], bf16, tag="pb")
                nc.sync.dma_start(pb[:], pd[:].to_broadcast((P, TC)))
                pb_ap = pb[:, :]
                # ctx_raw[:, ht] += sum_t p[t] * x[t]
                for ht in range(KT):
                    to = toutp.tile([P, TC], bf16, tag="to")
                    nc.vector.scalar_tensor_tensor(
                        out=to[:],
                        in0=et[:, ht, :],
                        scalar=1.0,
                        in1=pb_ap,
                        op0=Alu.mult,
                        op1=Alu.mult,
                        accum_out=ca[:, ht, tci : tci + 1],
                    )
            # batch row sum, then broadcast it to all 128 partitions with a
            # K=1 ones-matmul, reciprocal, scale the raw context, DMA out —
            # all per batch, so only the last batch's (tiny) chain is exposed.
            nc.vector.tensor_reduce(
                sums[row, NTC : NTC + 1],
                sums[row, 0:NTC],
                axis=mybir.AxisListType.X,
                op=Alu.add,
            )
            nc.vector.tensor_reduce(
                out_sb[:, b, :], ca[:], axis=mybir.AxisListType.X, op=Alu.add
            )
            psb = pep.tile([P, 1], f32, tag="pe", name="psb")
            nc.tensor.matmul(
                psb[:, :],
                ones_sb[row, :],
                sums[row, NTC : NTC + 1],
                start=True,
                stop=True,
                tile_position=(32 * b, 0),
                skip_group_check=True,
            )
            rec = toutp.tile([P, 1], f32, tag="rec")
            nc.vector.reciprocal(rec[:], psb[:, :])
            nc.vector.tensor_mul(
                out_sb[:, b, :],
                out_sb[:, b, :],
                rec[:].to_broadcast((P, OT)),
            )
            nc.sync.dma_start(
                out_e.rearrange("b (ht p) -> p b ht", p=P)[:, b, :],
                out_sb[:, b, :],
            )

    nc.finalize()
    return nc


def _prep_in_maps(enc_seq, enc_mask, dec_state, W_h, W_s, v):
    bf = ml_dtypes.bfloat16
    w_ht = np.ascontiguousarray(W_h.T).astype(bf)
    w_st = np.ascontiguousarray(W_s.T).astype(bf)
    v_in = np.ascontiguousarray(v.reshape(KT, P).T).astype(bf)
    in_maps = []
    for c in range(NCORES):
        sl = slice(c * BL, (c + 1) * BL)
        enc_t = np.ascontiguousarray(
            enc_seq[sl].transpose(0, 2, 1)
        ).astype(bf)
        maskb = np.where(enc_mask[sl] == 0, np.float32(NEG), np.float32(0.0)).astype(bf)
        dec_t = np.ascontiguousarray(dec_state[sl].T).astype(bf)
        in_maps.append(
            {
                "enc_t": enc_t,
                "maskb": maskb,
                "dec_t": dec_t,
                "w_ht": w_ht,
                "w_st": w_st,
                "v_in": v_in,
            }
        )
    return in_maps


def _run(inputs, trace=False):
    from concourse.bass_utils import run_bass_kernel_spmd

    if "nc" not in _CACHE:
        _CACHE["nc"] = _build()
    nc = _CACHE["nc"]
    in_maps = _prep_in_maps(**{k: np.asarray(v) for k, v in inputs.items()})
    res = run_bass_kernel_spmd(nc, in_maps, core_ids=list(range(NCORES)), trace=trace)
    out = np.concatenate([res.results[c]["out"] for c in range(NCORES)], axis=0)
    return out.astype(np.float32), res


def kernel(**inputs):
    out, _ = _run(inputs, trace=False)
    return out
